# revision 11
# baseline (speedup 1.0000x reference)
"""Trainium2 Bass kernel for nn_GATSkip_WeightShare (GAT message passing).

Strategy (8 NeuronCores, dst-node data parallel):
  - Nodes are partitioned into 8 contiguous ranges of 12500 (core = id // 12500),
    padded to 12544 = 98 blocks of 128 per core. Within a core, nodes are sorted
    ascending by in-degree so each 128-node block has near-uniform degree D_b;
    per-block edge slots are laid out k-major (slot (b,k) = column of 128 nodes'
    k-th in-neighbor).
  - A replicated node table [100352, 65] = [h | al] (h = x @ W, al = h @ a_src)
    lives in DRAM on every core. Per block, one indirect DMA gathers
    [128, D_b*65] rows by source-node id.
  - Attention is computed node-major: per-partition scalars (ar, z) broadcast
    along the free dim; exp(leaky_relu(...)) on the scalar engine with fused
    row-sum (accum_out) for z; the weighted sum is one broadcast multiply +
    strided reduce on the vector engine. No segment max is needed: softmax is
    shift-invariant and logits are O(10) here, so raw exp is safe. Padded slots
    get al += -1e9 so exp underflows to exactly 0.
  - Layer epilogue per block: gelu, PE-transpose, one matmul against
    [W | W@a_src | W@a_dst] producing next [h | al | ar] in one shot.
  - Cores exchange their 12544-row table slice via AllGather after each layer.
"""

import sys

sys.path.insert(0, "/opt/trn_rl_repo")

import numpy as np

import concourse.bass as bass
import concourse.tile as tile
from concourse import bacc, mybir
from concourse.bass_utils import run_bass_kernel_spmd
from concourse.masks import make_identity

P = 128
NCORES = 8
N = 100000
DIM = 64
NPC = N // NCORES          # 12500 nodes per core
NBLK = (NPC + P - 1) // P  # 98 blocks
NPCPAD = NBLK * P          # 12544
NROWS = NCORES * NPCPAD    # 100352 replicated-table rows
TW = DIM + 1               # table row: [h (64) | al]
F32 = mybir.dt.float32
I32 = mybir.dt.int32
NEG = -1.0e9

# test.py can flip these knobs
TRACE = False
LAST_RESULTS = None

_cache = {}


def _host_preprocess(edge_index):
    """Static graph metadata: per-core degree-sorted blocks, k-major gather
    indices, pad masks, permutation."""
    src_all = np.concatenate([edge_index[0].astype(np.int64), np.arange(N)])
    dst_all = np.concatenate([edge_index[1].astype(np.int64), np.arange(N)])

    deg = np.bincount(dst_all, minlength=N)

    sorted_nodes_per_core = []
    degs_per_core = []
    for c in range(NCORES):
        nodes = np.arange(c * NPC, (c + 1) * NPC)
        order = np.argsort(deg[nodes], kind="stable")  # ascending degree
        sn = nodes[order]
        sorted_nodes_per_core.append(sn)
        dp = np.concatenate([deg[sn], np.zeros(NPCPAD - NPC, np.int64)])
        degs_per_core.append(dp)

    # shared per-block neighbor count D_b = max over cores (SPMD: same program)
    Db = np.stack([d.reshape(NBLK, P).max(axis=1) for d in degs_per_core]).max(axis=0)
    Db = np.maximum(Db, 1).astype(np.int64)
    C = int(Db.sum())

    # global permuted row id for each original node
    gid = np.empty(N, np.int64)
    for c in range(NCORES):
        gid[sorted_nodes_per_core[c]] = c * NPCPAD + np.arange(NPC)

    order_e = np.argsort(dst_all, kind="stable")
    src_sorted = src_all[order_e]
    dst_sorted = dst_all[order_e]
    starts = np.searchsorted(dst_sorted, np.arange(N))
    ends = np.searchsorted(dst_sorted, np.arange(N) + 1)
    gsrc_sorted = gid[src_sorted]

    col_of_block = np.concatenate([[0], np.cumsum(Db)[:-1]])

    srcidx_per_core = []
    maskneg_per_core = []
    for c in range(NCORES):
        sn = sorted_nodes_per_core[c]
        dp = degs_per_core[c]
        srcidx = np.zeros((P, C), np.int32)
        maskneg = np.full((P, C), NEG, np.float32)
        for b in range(NBLK):
            cb = int(col_of_block[b])
            D = int(Db[b])
            for d in range(P):
                li = b * P + d
                if li >= NPC:
                    continue
                node = sn[li]
                s, e = int(starts[node]), int(ends[node])
                k = e - s
                srcidx[d, cb:cb + k] = gsrc_sorted[s:e]
                maskneg[d, cb:cb + k] = 0.0
        srcidx_per_core.append(srcidx)
        maskneg_per_core.append(maskneg)

    return {
        "sorted_nodes": sorted_nodes_per_core,
        "Db": Db,
        "C": C,
        "col_of_block": col_of_block,
        "srcidx": srcidx_per_core,
        "maskneg": maskneg_per_core,
    }


def _build_program(Db, C, col_of_block):
    """Build + compile the SPMD Bass program (same for all cores)."""
    nc = bacc.Bacc(None, target_bir_lowering=False)

    x0T_d = nc.dram_tensor("x0T", [DIM, NPCPAD], F32, kind="ExternalInput")
    srcidx_d = nc.dram_tensor("srcidx", [P, C], I32, kind="ExternalInput")
    maskneg_d = nc.dram_tensor("maskneg", [P, C], F32, kind="ExternalInput")
    rhs_std_d = nc.dram_tensor("rhs_std", [DIM, TW + 1], F32, kind="ExternalInput")
    rhs_sklo_d = nc.dram_tensor("rhs_sklo", [DIM, TW + 1], F32, kind="ExternalInput")
    rhs_skhi_d = nc.dram_tensor("rhs_skhi", [DIM, TW + 1], F32, kind="ExternalInput")
    bstd_d = nc.dram_tensor("bstd", [P, DIM], F32, kind="ExternalInput")
    bskip_d = nc.dram_tensor("bskip", [P, DIM], F32, kind="ExternalInput")
    out_d = nc.dram_tensor("out", [NPCPAD, DIM], F32, kind="ExternalOutput")

    groups = [list(range(NCORES))]

    with tile.TileContext(nc) as tc:
        with tc.tile_pool(name="dram", bufs=1, space="DRAM") as dramp, \
             tc.tile_pool(name="res", bufs=1) as res, \
             tc.tile_pool(name="xg", bufs=3) as xgp, \
             tc.tile_pool(name="prodp", bufs=2) as prodp, \
             tc.tile_pool(name="small", bufs=6) as small, \
             tc.tile_pool(name="stage", bufs=4) as stagep, \
             tc.tile_pool(name="psum", bufs=4, space="PSUM") as psum:

            table_stage = dramp.tile([NPCPAD, TW], F32, name="table_stage")
            # one Shared AllGather output per layer (Shared DRAM tensors must
            # have a single writer instruction)
            table_fulls = [dramp.tile([NROWS, TW], F32, name=f"table_full{i}",
                                      addr_space="Shared") for i in range(5)]

            # ---- resident tiles ----
            srcidx_t = res.tile([P, C], I32)
            nc.sync.dma_start(srcidx_t[:], srcidx_d[:])
            maskneg_t = res.tile([P, C], F32)
            nc.sync.dma_start(maskneg_t[:], maskneg_d[:])
            rhs_std_t = res.tile([DIM, TW + 1], F32)
            nc.sync.dma_start(rhs_std_t[:], rhs_std_d[:])
            rhs_sklo_t = res.tile([DIM, TW + 1], F32)
            nc.sync.dma_start(rhs_sklo_t[:], rhs_sklo_d[:])
            rhs_skhi_t = res.tile([DIM, TW + 1], F32)
            nc.sync.dma_start(rhs_skhi_t[:], rhs_skhi_d[:])
            bstd_t = res.tile([P, DIM], F32)
            nc.sync.dma_start(bstd_t[:], bstd_d[:])
            bskip_t = res.tile([P, DIM], F32)
            nc.sync.dma_start(bskip_t[:], bskip_d[:])
            ident_t = res.tile([P, P], F32)
            make_identity(nc, ident_t[:])

            ar_a = res.tile([P, NBLK], F32)
            ar_b = res.tile([P, NBLK], F32)
            u_all = res.tile([P, NBLK * DIM], F32)
            rz_all = res.tile([P, NBLK], F32)
            xn_all = res.tile([P, NBLK * DIM], F32)

            def epilogue_block(b, lhsT_ap, rhs_t, ar_dst, second_lhsT=None,
                               second_rhs=None):
                """psum = lhsT.T @ rhs (+ second); write table_stage rows +
                ar_dst column."""
                mm_ps = psum.tile([P, TW + 1], F32, tag="mm", space="PSUM")
                nc.tensor.matmul(mm_ps[:], lhsT=lhsT_ap, rhs=rhs_t[:],
                                 start=True, stop=second_lhsT is None)
                if second_lhsT is not None:
                    nc.tensor.matmul(mm_ps[:], lhsT=second_lhsT, rhs=second_rhs[:],
                                     start=False, stop=True)
                stg = stagep.tile([P, TW + 1], F32, tag="stg")
                nc.vector.tensor_copy(stg[:], mm_ps[:])
                nc.sync.dma_start(table_stage[b * P:(b + 1) * P, :], stg[:, :TW])
                nc.vector.tensor_copy(ar_dst[:, b:b + 1], stg[:, TW:TW + 1])

            def attention_layer(ar_cur, table_full):
                """Phase A over all blocks: fills u_all (unnormalized agg) and
                rz_all (1/z)."""
                for b in range(NBLK):
                    cb = int(col_of_block[b])
                    D = int(Db[b])
                    Xg = xgp.tile([P, D * TW], F32, tag="xg")
                    # HW constraint: one offset per dest partition-run, so one
                    # indirect DMA per neighbor column.
                    for k in range(D):
                        nc.gpsimd.indirect_dma_start(
                            out=Xg[:, k * TW:(k + 1) * TW], out_offset=None,
                            in_=table_full[:],
                            in_offset=bass.IndirectOffsetOnAxis(
                                ap=srcidx_t[:, cb + k:cb + k + 1], axis=0))
                    al_view = Xg[:].rearrange("p (d w) -> p d w", w=TW)[:, :, DIM]
                    # leaky_relu(pre, 0.2) = 0.6*pre + 0.4*|pre|; HW Lrelu has a
                    # baked 0.01 slope (ignores alpha), so compose via Abs.
                    pre = small.tile([P, D], F32, tag="pre")
                    nc.vector.tensor_scalar_add(pre[:], al_view, ar_cur[:, b:b + 1])
                    absv = small.tile([P, D], F32, tag="absv")
                    nc.scalar.activation(absv[:], al_view,
                                         mybir.ActivationFunctionType.Abs,
                                         bias=ar_cur[:, b:b + 1], scale=1.0)
                    m1 = small.tile([P, D], F32, tag="m1")
                    nc.vector.scalar_tensor_tensor(
                        m1[:], in0=absv[:], scalar=2.0 / 3.0, in1=pre[:],
                        op0=mybir.AluOpType.mult, op1=mybir.AluOpType.add)
                    # mask AFTER: exp(0.6*(m1 - 1e9)) == 0 for pads
                    lg_m = small.tile([P, D], F32, tag="lgm")
                    nc.vector.tensor_add(lg_m[:], m1[:], maskneg_t[:, cb:cb + D])
                    e_t = small.tile([P, D], F32, tag="e")
                    z_t = small.tile([P, 1], F32, tag="z")
                    nc.scalar.activation(e_t[:], lg_m[:],
                                         mybir.ActivationFunctionType.Exp,
                                         scale=0.6, accum_out=z_t[:])
                    nc.vector.reciprocal(rz_all[:, b:b + 1], z_t[:])
                    prod = prodp.tile([P, DIM * D], F32, tag="prod")
                    prod_v = prod[:].rearrange("p (w d) -> p w d", d=D)
                    Xg_v = Xg[:].rearrange("p (d w) -> p w d", w=TW)[:, :DIM, :]
                    nc.vector.tensor_mul(prod_v, Xg_v,
                                         e_t[:].unsqueeze(1).to_broadcast([P, DIM, D]))
                    nc.vector.tensor_reduce(u_all[:, b * DIM:(b + 1) * DIM], prod_v,
                                            axis=mybir.AxisListType.X,
                                            op=mybir.AluOpType.add)

            def allgather(table_full):
                nc.gpsimd.collective_compute(
                    "AllGather", mybir.AluOpType.bypass, replica_groups=groups,
                    ins=[table_stage[:]], outs=[table_full[:]])

            def load_x0T_blk(b):
                t = stagep.tile([DIM, P], F32, tag="x0Tblk")
                nc.sync.dma_start(t[:], x0T_d[:, b * P:(b + 1) * P])
                return t

            # ---- layer-1 prologue: table rows for own nodes from x0 ----
            for b in range(NBLK):
                x0blk = load_x0T_blk(b)
                epilogue_block(b, x0blk[:], rhs_std_t, ar_a)
            allgather(table_fulls[0])

            ar_cur, ar_nxt = ar_a, ar_b
            for layer in range(4):
                attention_layer(ar_cur, table_fulls[layer])
                # phase B: x_next = gelu(u * rz + b) for all blocks
                for b in range(NBLK):
                    gin = stagep.tile([P, DIM], F32, tag="gin")
                    nc.vector.scalar_tensor_tensor(
                        gin[:], in0=u_all[:, b * DIM:(b + 1) * DIM],
                        scalar=rz_all[:, b:b + 1], in1=bstd_t[:],
                        op0=mybir.AluOpType.mult, op1=mybir.AluOpType.add)
                    nc.scalar.activation(xn_all[:, b * DIM:(b + 1) * DIM], gin[:],
                                         mybir.ActivationFunctionType.Gelu)
                # phase C: epilogue matmuls -> table_stage (+ skip extra matmul
                # at layer 3)
                for b in range(NBLK):
                    tr_ps = psum.tile([DIM, P], F32, tag="tr", space="PSUM")
                    nc.tensor.transpose(out=tr_ps[:],
                                        in_=xn_all[:, b * DIM:(b + 1) * DIM],
                                        identity=ident_t[:])
                    xnT = stagep.tile([DIM, P], F32, tag="xnT")
                    nc.vector.tensor_copy(xnT[:], tr_ps[:])
                    if layer < 3:
                        epilogue_block(b, xnT[:], rhs_std_t, ar_nxt)
                    else:
                        x0blk = load_x0T_blk(b)
                        epilogue_block(b, xnT[:], rhs_skhi_t, ar_nxt,
                                       second_lhsT=x0blk[:],
                                       second_rhs=rhs_sklo_t)
                allgather(table_fulls[layer + 1])
                ar_cur, ar_nxt = ar_nxt, ar_cur

            # ---- skip layer: attention + final output (no gelu) ----
            attention_layer(ar_cur, table_fulls[4])
            for b in range(NBLK):
                ou = stagep.tile([P, DIM], F32, tag="ou")
                nc.vector.scalar_tensor_tensor(
                    ou[:], in0=u_all[:, b * DIM:(b + 1) * DIM],
                    scalar=rz_all[:, b:b + 1], in1=bskip_t[:],
                    op0=mybir.AluOpType.mult, op1=mybir.AluOpType.add)
                nc.sync.dma_start(out_d[b * P:(b + 1) * P, :], ou[:])

    nc.compile()
    return nc


def kernel(**inputs):
    global LAST_RESULTS
    x = np.ascontiguousarray(np.asarray(inputs["x"], dtype=np.float32))
    edge_index = np.asarray(inputs["edge_index"])
    W_std = np.asarray(inputs["W_std"], np.float32)
    a_src_std = np.asarray(inputs["a_src_std"], np.float32)
    a_dst_std = np.asarray(inputs["a_dst_std"], np.float32)
    b_std = np.asarray(inputs["b_std"], np.float32)
    W_skip = np.asarray(inputs["W_skip"], np.float32)
    a_src_skip = np.asarray(inputs["a_src_skip"], np.float32)
    a_dst_skip = np.asarray(inputs["a_dst_skip"], np.float32)
    b_skip = np.asarray(inputs["b_skip"], np.float32)

    key = hash(edge_index.tobytes())
    if key not in _cache:
        meta = _host_preprocess(edge_index)
        prog = _build_program(meta["Db"], meta["C"], meta["col_of_block"])
        _cache[key] = (meta, prog)
    meta, prog = _cache[key]

    # weight-derived host constants
    def rhs_cat(W, wa, wb):
        return np.ascontiguousarray(
            np.concatenate([W, wa[:, None], wb[:, None]], axis=1).astype(np.float32))

    wa_std = W_std @ a_src_std
    wb_std = W_std @ a_dst_std
    wa_skip = W_skip @ a_src_skip
    wb_skip = W_skip @ a_dst_skip
    rhs_std = rhs_cat(W_std, wa_std, wb_std)
    rhs_sklo = rhs_cat(W_skip[:DIM], wa_skip[:DIM], wb_skip[:DIM])
    rhs_skhi = rhs_cat(W_skip[DIM:], wa_skip[DIM:], wb_skip[DIM:])
    bstd_b = np.ascontiguousarray(np.broadcast_to(b_std, (P, DIM)).astype(np.float32))
    bskip_b = np.ascontiguousarray(np.broadcast_to(b_skip, (P, DIM)).astype(np.float32))

    in_maps = []
    for c in range(NCORES):
        sn = meta["sorted_nodes"][c]
        x0T = np.zeros((DIM, NPCPAD), np.float32)
        x0T[:, :NPC] = x[sn].T
        in_maps.append({
            "x0T": x0T,
            "srcidx": meta["srcidx"][c],
            "maskneg": meta["maskneg"][c],
            "rhs_std": rhs_std,
            "rhs_sklo": rhs_sklo,
            "rhs_skhi": rhs_skhi,
            "bstd": bstd_b,
            "bskip": bskip_b,
        })

    res = run_bass_kernel_spmd(prog, in_maps, core_ids=list(range(NCORES)),
                               trace=TRACE)
    LAST_RESULTS = res

    out = np.empty((N, DIM), np.float32)
    for c in range(NCORES):
        sn = meta["sorted_nodes"][c]
        out[sn] = res.results[c]["out"][:NPC]
    return out


# revision 14
# speedup vs baseline: 1.0792x; 1.0792x over previous
"""Trainium2 Bass kernel for nn_GATSkip_WeightShare (GAT message passing).

Strategy (8 NeuronCores, dst-node data parallel):
  - Nodes are partitioned into 8 contiguous ranges of 12500 (core = id // 12500),
    padded to 12544 = 98 blocks of 128 per core. Within a core, nodes are sorted
    ascending by in-degree so each 128-node block has near-uniform degree D_b;
    per-block edge slots are laid out k-major (slot (b,k) = column of 128 nodes'
    k-th in-neighbor).
  - A replicated node table [100352, 65] = [h | al] (h = x @ W, al = h @ a_src)
    lives in DRAM on every core. Per block, one indirect DMA gathers
    [128, D_b*65] rows by source-node id.
  - Attention is computed node-major: per-partition scalars (ar, z) broadcast
    along the free dim; exp(leaky_relu(...)) on the scalar engine with fused
    row-sum (accum_out) for z; the weighted sum is one broadcast multiply +
    strided reduce on the vector engine. No segment max is needed: softmax is
    shift-invariant and logits are O(10) here, so raw exp is safe. Padded slots
    get al += -1e9 so exp underflows to exactly 0.
  - Layer epilogue per block: gelu, PE-transpose, one matmul against
    [W | W@a_src | W@a_dst] producing next [h | al | ar] in one shot.
  - Cores exchange their 12544-row table slice via AllGather after each layer.
"""

import sys

sys.path.insert(0, "/opt/trn_rl_repo")

import numpy as np

import concourse.bass as bass
import concourse.tile as tile
from concourse import bacc, mybir
from concourse.bass_utils import run_bass_kernel_spmd
from concourse.masks import make_identity

P = 128
NCORES = 8
N = 100000
DIM = 64
NPC = N // NCORES          # 12500 nodes per core
NBLK = (NPC + P - 1) // P  # 98 blocks
NPCPAD = NBLK * P          # 12544
NROWS = NCORES * NPCPAD    # 100352 replicated-table rows
TW = DIM + 1               # table row: [h (64) | al]
F32 = mybir.dt.float32
I32 = mybir.dt.int32
NEG = -1.0e9

# test.py can flip these knobs
TRACE = False
LAST_RESULTS = None

_cache = {}


def _host_preprocess(edge_index):
    """Static graph metadata: per-core degree-sorted blocks, k-major gather
    indices, pad masks, permutation."""
    # self-loops FIRST so each real node's k=0 neighbor is itself: that column
    # is then own-block rows of the table, fetched by one contiguous HWDGE DMA
    # instead of a Pool-serial indirect gather.
    src_all = np.concatenate([np.arange(N), edge_index[0].astype(np.int64)])
    dst_all = np.concatenate([np.arange(N), edge_index[1].astype(np.int64)])

    deg = np.bincount(dst_all, minlength=N)

    sorted_nodes_per_core = []
    degs_per_core = []
    for c in range(NCORES):
        nodes = np.arange(c * NPC, (c + 1) * NPC)
        order = np.argsort(deg[nodes], kind="stable")  # ascending degree
        sn = nodes[order]
        sorted_nodes_per_core.append(sn)
        dp = np.concatenate([deg[sn], np.zeros(NPCPAD - NPC, np.int64)])
        degs_per_core.append(dp)

    # shared per-block neighbor count D_b = max over cores (SPMD: same program)
    Db = np.stack([d.reshape(NBLK, P).max(axis=1) for d in degs_per_core]).max(axis=0)
    Db = np.maximum(Db, 1).astype(np.int64)
    C = int(Db.sum())

    # global permuted row id for each original node
    gid = np.empty(N, np.int64)
    for c in range(NCORES):
        gid[sorted_nodes_per_core[c]] = c * NPCPAD + np.arange(NPC)

    order_e = np.argsort(dst_all, kind="stable")
    src_sorted = src_all[order_e]
    dst_sorted = dst_all[order_e]
    starts = np.searchsorted(dst_sorted, np.arange(N))
    ends = np.searchsorted(dst_sorted, np.arange(N) + 1)
    gsrc_sorted = gid[src_sorted]

    col_of_block = np.concatenate([[0], np.cumsum(Db)[:-1]])

    srcidx_per_core = []
    maskneg_per_core = []
    for c in range(NCORES):
        sn = sorted_nodes_per_core[c]
        dp = degs_per_core[c]
        srcidx = np.zeros((P, C), np.int32)
        maskneg = np.full((P, C), NEG, np.float32)
        for b in range(NBLK):
            cb = int(col_of_block[b])
            D = int(Db[b])
            for d in range(P):
                li = b * P + d
                if li >= NPC:
                    continue
                node = sn[li]
                s, e = int(starts[node]), int(ends[node])
                k = e - s
                srcidx[d, cb:cb + k] = gsrc_sorted[s:e]
                maskneg[d, cb:cb + k] = 0.0
        srcidx_per_core.append(srcidx)
        maskneg_per_core.append(maskneg)

    return {
        "sorted_nodes": sorted_nodes_per_core,
        "Db": Db,
        "C": C,
        "col_of_block": col_of_block,
        "srcidx": srcidx_per_core,
        "maskneg": maskneg_per_core,
    }


def _build_program(Db, C, col_of_block):
    """Build + compile the SPMD Bass program (same for all cores)."""
    nc = bacc.Bacc(None, target_bir_lowering=False)

    x0T_d = nc.dram_tensor("x0T", [DIM, NPCPAD], F32, kind="ExternalInput")
    srcidx_d = nc.dram_tensor("srcidx", [P, C], I32, kind="ExternalInput")
    maskneg_d = nc.dram_tensor("maskneg", [P, C], F32, kind="ExternalInput")
    rhs_std_d = nc.dram_tensor("rhs_std", [DIM, TW + 1], F32, kind="ExternalInput")
    rhs_sklo_d = nc.dram_tensor("rhs_sklo", [DIM, TW + 1], F32, kind="ExternalInput")
    rhs_skhi_d = nc.dram_tensor("rhs_skhi", [DIM, TW + 1], F32, kind="ExternalInput")
    bstd_d = nc.dram_tensor("bstd", [P, DIM], F32, kind="ExternalInput")
    bskip_d = nc.dram_tensor("bskip", [P, DIM], F32, kind="ExternalInput")
    out_d = nc.dram_tensor("out", [NPCPAD, DIM], F32, kind="ExternalOutput")

    groups = [list(range(NCORES))]

    with tile.TileContext(nc) as tc:
        with tc.tile_pool(name="dram", bufs=1, space="DRAM") as dramp, \
             tc.tile_pool(name="res", bufs=1) as res, \
             tc.tile_pool(name="xg", bufs=3) as xgp, \
             tc.tile_pool(name="prodp", bufs=2) as prodp, \
             tc.tile_pool(name="small", bufs=6) as small, \
             tc.tile_pool(name="stage", bufs=4) as stagep, \
             tc.tile_pool(name="psum", bufs=4, space="PSUM") as psum:

            table_stage = dramp.tile([NPCPAD, TW], F32, name="table_stage")
            # one Shared AllGather output per layer (Shared DRAM tensors must
            # have a single writer instruction)
            table_fulls = [dramp.tile([NROWS, TW], F32, name=f"table_full{i}",
                                      addr_space="Shared") for i in range(5)]

            # ---- resident tiles ----
            srcidx_t = res.tile([P, C], I32)
            nc.sync.dma_start(srcidx_t[:], srcidx_d[:])
            maskneg_t = res.tile([P, C], F32)
            nc.sync.dma_start(maskneg_t[:], maskneg_d[:])
            rhs_std_t = res.tile([DIM, TW + 1], F32)
            nc.sync.dma_start(rhs_std_t[:], rhs_std_d[:])
            rhs_sklo_t = res.tile([DIM, TW + 1], F32)
            nc.sync.dma_start(rhs_sklo_t[:], rhs_sklo_d[:])
            rhs_skhi_t = res.tile([DIM, TW + 1], F32)
            nc.sync.dma_start(rhs_skhi_t[:], rhs_skhi_d[:])
            bstd_t = res.tile([P, DIM], F32)
            nc.sync.dma_start(bstd_t[:], bstd_d[:])
            bskip_t = res.tile([P, DIM], F32)
            nc.sync.dma_start(bskip_t[:], bskip_d[:])
            ident_t = res.tile([P, P], F32)
            make_identity(nc, ident_t[:])

            ar_a = res.tile([P, NBLK], F32)
            ar_b = res.tile([P, NBLK], F32)
            u_all = res.tile([P, NBLK * DIM], F32)
            rz_all = res.tile([P, NBLK], F32)
            xn_all = res.tile([P, NBLK * DIM], F32)

            def epilogue_block(b, lhsT_ap, rhs_t, ar_dst, second_lhsT=None,
                               second_rhs=None):
                """psum = lhsT.T @ rhs (+ second); write table_stage rows +
                ar_dst column."""
                mm_ps = psum.tile([P, TW + 1], F32, tag="mm", space="PSUM")
                nc.tensor.matmul(mm_ps[:], lhsT=lhsT_ap, rhs=rhs_t[:],
                                 start=True, stop=second_lhsT is None)
                if second_lhsT is not None:
                    nc.tensor.matmul(mm_ps[:], lhsT=second_lhsT, rhs=second_rhs[:],
                                     start=False, stop=True)
                stg = stagep.tile([P, TW + 1], F32, tag="stg")
                nc.vector.tensor_copy(stg[:], mm_ps[:])
                nc.sync.dma_start(table_stage[b * P:(b + 1) * P, :], stg[:, :TW])
                nc.vector.tensor_copy(ar_dst[:, b:b + 1], stg[:, TW:TW + 1])

            def attention_layer(ar_cur, table_full):
                """Phase A over all blocks: fills u_all (unnormalized agg) and
                rz_all (1/z)."""
                for b in range(NBLK):
                    cb = int(col_of_block[b])
                    D = int(Db[b])
                    Xg = xgp.tile([P, D * TW], F32, tag="xg")
                    # k=0 is the self-loop: own block rows live in table_stage
                    # (core-local, same address on every core) -> plain HWDGE DMA
                    nc.sync.dma_start(
                        out=Xg[:, 0:TW],
                        in_=table_stage[b * P:(b + 1) * P, :])
                    # HW constraint: one offset per dest partition-run, so one
                    # indirect DMA per remaining neighbor column.
                    for k in range(1, D):
                        nc.gpsimd.indirect_dma_start(
                            out=Xg[:, k * TW:(k + 1) * TW], out_offset=None,
                            in_=table_full[:],
                            in_offset=bass.IndirectOffsetOnAxis(
                                ap=srcidx_t[:, cb + k:cb + k + 1], axis=0))
                    al_view = Xg[:].rearrange("p (d w) -> p d w", w=TW)[:, :, DIM]
                    # leaky_relu(pre, 0.2) = 0.6*pre + 0.4*|pre|; HW Lrelu has a
                    # baked 0.01 slope (ignores alpha), so compose via Abs.
                    pre = small.tile([P, D], F32, tag="pre")
                    nc.vector.tensor_scalar_add(pre[:], al_view, ar_cur[:, b:b + 1])
                    absv = small.tile([P, D], F32, tag="absv")
                    nc.scalar.activation(absv[:], al_view,
                                         mybir.ActivationFunctionType.Abs,
                                         bias=ar_cur[:, b:b + 1], scale=1.0)
                    m1 = small.tile([P, D], F32, tag="m1")
                    nc.vector.scalar_tensor_tensor(
                        m1[:], in0=absv[:], scalar=2.0 / 3.0, in1=pre[:],
                        op0=mybir.AluOpType.mult, op1=mybir.AluOpType.add)
                    # mask AFTER: exp(0.6*(m1 - 1e9)) == 0 for pads
                    lg_m = small.tile([P, D], F32, tag="lgm")
                    nc.vector.tensor_add(lg_m[:], m1[:], maskneg_t[:, cb:cb + D])
                    e_t = small.tile([P, D], F32, tag="e")
                    z_t = small.tile([P, 1], F32, tag="z")
                    nc.scalar.activation(e_t[:], lg_m[:],
                                         mybir.ActivationFunctionType.Exp,
                                         scale=0.6, accum_out=z_t[:])
                    nc.vector.reciprocal(rz_all[:, b:b + 1], z_t[:])
                    prod = prodp.tile([P, DIM * D], F32, tag="prod")
                    prod_v = prod[:].rearrange("p (w d) -> p w d", d=D)
                    Xg_v = Xg[:].rearrange("p (d w) -> p w d", w=TW)[:, :DIM, :]
                    nc.vector.tensor_mul(prod_v, Xg_v,
                                         e_t[:].unsqueeze(1).to_broadcast([P, DIM, D]))
                    nc.vector.tensor_reduce(u_all[:, b * DIM:(b + 1) * DIM], prod_v,
                                            axis=mybir.AxisListType.X,
                                            op=mybir.AluOpType.add)

            def allgather(table_full):
                nc.gpsimd.collective_compute(
                    "AllGather", mybir.AluOpType.bypass, replica_groups=groups,
                    ins=[table_stage[:]], outs=[table_full[:]])

            def load_x0T_blk(b):
                t = stagep.tile([DIM, P], F32, tag="x0Tblk")
                nc.sync.dma_start(t[:], x0T_d[:, b * P:(b + 1) * P])
                return t

            # ---- layer-1 prologue: table rows for own nodes from x0 ----
            for b in range(NBLK):
                x0blk = load_x0T_blk(b)
                epilogue_block(b, x0blk[:], rhs_std_t, ar_a)
            allgather(table_fulls[0])

            ar_cur, ar_nxt = ar_a, ar_b
            for layer in range(4):
                attention_layer(ar_cur, table_fulls[layer])
                # phase B: x_next = gelu(u * rz + b) for all blocks
                for b in range(NBLK):
                    gin = stagep.tile([P, DIM], F32, tag="gin")
                    nc.vector.scalar_tensor_tensor(
                        gin[:], in0=u_all[:, b * DIM:(b + 1) * DIM],
                        scalar=rz_all[:, b:b + 1], in1=bstd_t[:],
                        op0=mybir.AluOpType.mult, op1=mybir.AluOpType.add)
                    nc.scalar.activation(xn_all[:, b * DIM:(b + 1) * DIM], gin[:],
                                         mybir.ActivationFunctionType.Gelu)
                # phase C: epilogue matmuls -> table_stage (+ skip extra matmul
                # at layer 3)
                for b in range(NBLK):
                    tr_ps = psum.tile([DIM, P], F32, tag="tr", space="PSUM")
                    nc.tensor.transpose(out=tr_ps[:],
                                        in_=xn_all[:, b * DIM:(b + 1) * DIM],
                                        identity=ident_t[:])
                    xnT = stagep.tile([DIM, P], F32, tag="xnT")
                    nc.vector.tensor_copy(xnT[:], tr_ps[:])
                    if layer < 3:
                        epilogue_block(b, xnT[:], rhs_std_t, ar_nxt)
                    else:
                        x0blk = load_x0T_blk(b)
                        epilogue_block(b, xnT[:], rhs_skhi_t, ar_nxt,
                                       second_lhsT=x0blk[:],
                                       second_rhs=rhs_sklo_t)
                allgather(table_fulls[layer + 1])
                ar_cur, ar_nxt = ar_nxt, ar_cur

            # ---- skip layer: attention + final output (no gelu) ----
            attention_layer(ar_cur, table_fulls[4])
            for b in range(NBLK):
                ou = stagep.tile([P, DIM], F32, tag="ou")
                nc.vector.scalar_tensor_tensor(
                    ou[:], in0=u_all[:, b * DIM:(b + 1) * DIM],
                    scalar=rz_all[:, b:b + 1], in1=bskip_t[:],
                    op0=mybir.AluOpType.mult, op1=mybir.AluOpType.add)
                nc.sync.dma_start(out_d[b * P:(b + 1) * P, :], ou[:])

    nc.compile()
    return nc


def kernel(**inputs):
    global LAST_RESULTS
    x = np.ascontiguousarray(np.asarray(inputs["x"], dtype=np.float32))
    edge_index = np.asarray(inputs["edge_index"])
    W_std = np.asarray(inputs["W_std"], np.float32)
    a_src_std = np.asarray(inputs["a_src_std"], np.float32)
    a_dst_std = np.asarray(inputs["a_dst_std"], np.float32)
    b_std = np.asarray(inputs["b_std"], np.float32)
    W_skip = np.asarray(inputs["W_skip"], np.float32)
    a_src_skip = np.asarray(inputs["a_src_skip"], np.float32)
    a_dst_skip = np.asarray(inputs["a_dst_skip"], np.float32)
    b_skip = np.asarray(inputs["b_skip"], np.float32)

    key = hash(edge_index.tobytes())
    if key not in _cache:
        meta = _host_preprocess(edge_index)
        prog = _build_program(meta["Db"], meta["C"], meta["col_of_block"])
        _cache[key] = (meta, prog)
    meta, prog = _cache[key]

    # weight-derived host constants
    def rhs_cat(W, wa, wb):
        return np.ascontiguousarray(
            np.concatenate([W, wa[:, None], wb[:, None]], axis=1).astype(np.float32))

    wa_std = W_std @ a_src_std
    wb_std = W_std @ a_dst_std
    wa_skip = W_skip @ a_src_skip
    wb_skip = W_skip @ a_dst_skip
    rhs_std = rhs_cat(W_std, wa_std, wb_std)
    rhs_sklo = rhs_cat(W_skip[:DIM], wa_skip[:DIM], wb_skip[:DIM])
    rhs_skhi = rhs_cat(W_skip[DIM:], wa_skip[DIM:], wb_skip[DIM:])
    bstd_b = np.ascontiguousarray(np.broadcast_to(b_std, (P, DIM)).astype(np.float32))
    bskip_b = np.ascontiguousarray(np.broadcast_to(b_skip, (P, DIM)).astype(np.float32))

    in_maps = []
    for c in range(NCORES):
        sn = meta["sorted_nodes"][c]
        x0T = np.zeros((DIM, NPCPAD), np.float32)
        x0T[:, :NPC] = x[sn].T
        in_maps.append({
            "x0T": x0T,
            "srcidx": meta["srcidx"][c],
            "maskneg": meta["maskneg"][c],
            "rhs_std": rhs_std,
            "rhs_sklo": rhs_sklo,
            "rhs_skhi": rhs_skhi,
            "bstd": bstd_b,
            "bskip": bskip_b,
        })

    res = run_bass_kernel_spmd(prog, in_maps, core_ids=list(range(NCORES)),
                               trace=TRACE)
    LAST_RESULTS = res

    out = np.empty((N, DIM), np.float32)
    for c in range(NCORES):
        sn = meta["sorted_nodes"][c]
        out[sn] = res.results[c]["out"][:NPC]
    return out
